# revision 4
# baseline (speedup 1.0000x reference)
"""Trainium2 Bass kernel for a post-LN transformer decoder layer (v2).

Sharding: 8 cores = 4 batches x 2 token-halves. Transposed layout
[d_model, tokens]; matmul operands bf16 (fp32 PSUM accumulate).

v2 restructure for PE p-state + engine overlap:
- Merged [128,1024] score PSUM tile per k-tile (both heads), ONE exp per
  k-tile, software-pipelined PV (scores of kt issued before PV of kt-1).
- Two filler queues interleave independent projection matmuls into the
  ACT-bound attention stream so the tensor engine never idles.
- Out-proj fused: attn(hp) is consumed immediately by 16 per-hp out-proj
  matmuls accumulated into an SBUF f32 tile; doubles as attention filler.
- Softmax normalize: per-hp batched reciprocal ([4,512] via DMA row
  gather; reciprocal cost is per-free-element, partitions are free).
- Residuals streamed from DRAM at z-assembly time; x1 spilled to DRAM
  across the cross-attention phase (SBUF pressure).
"""

import sys

sys.path.insert(0, "/opt/trn_rl_repo")

from collections import deque

import numpy as np
import ml_dtypes

import concourse.bass as bass
import concourse.tile as tile
from concourse import mybir
from concourse.bass_utils import run_bass_kernel_spmd

BF16 = mybir.dt.bfloat16
F32 = mybir.dt.float32
AF = mybir.ActivationFunctionType

D = 1024
DFF = 4096
B, S = 4, 2048
NCORES = 8
QL = 1024
EPS = 1e-6
MT = 8
FT = 32
SCALE = 0.125
SLOT_NKT = (8, 16)

V2_ROW0 = [0, 1024, 1536, 512]


def legalize_waits(nc, max_waits=1):
    nid = 0
    for fn in nc.m.functions:
        for bb in fn.blocks:
            new = []
            changed = False
            for inst in bb.instructions:
                si = inst.sync_info
                if si is not None and si.on_wait and len(si.on_wait) > max_waits:
                    waits = list(si.on_wait)
                    for w in waits[:-max_waits]:
                        nid += 1
                        nop = mybir.InstNoOp(name=f"I-waitfix-{nid}", ins=[], outs=[])
                        nop.engine = inst.engine
                        nop.sync_info = mybir.SyncInfo(on_wait=[w], on_update=[])
                        new.append(nop)
                    inst.sync_info = mybir.SyncInfo(
                        on_wait=waits[-max_waits:], on_update=list(si.on_update)
                    )
                    changed = True
                new.append(inst)
            if changed:
                bb.instructions = new


def build_nc():
    nc = bass.Bass(num_devices=NCORES)

    xT = nc.dram_tensor("xT", [D, S], BF16, kind="ExternalInput")
    xTq = nc.dram_tensor("xTq", [D, QL], BF16, kind="ExternalInput")
    encT = nc.dram_tensor("encT", [D, S], BF16, kind="ExternalInput")
    encTq = nc.dram_tensor("encTq", [D, QL], BF16, kind="ExternalInput")
    wd = {}
    for name in ("wq1", "wk1", "wv1", "wo1", "wq2", "wk2", "wv2", "wo2"):
        wd[name] = nc.dram_tensor(name, [D, D], BF16, kind="ExternalInput")
    wd["wf1"] = nc.dram_tensor("wf1", [D, DFF], BF16, kind="ExternalInput")
    wd["wf2"] = nc.dram_tensor("wf2", [DFF, D], BF16, kind="ExternalInput")
    bias_specs = (
        ("cq1", 8), ("ck1", 8), ("co1", 8), ("cq2", 8), ("ck2", 8), ("co2", 8),
        ("cf1", 32), ("cf2", 8),
        ("g1", 8), ("be1", 8), ("g2", 8), ("be2", 8), ("g3", 8), ("be3", 8),
    )
    bcd = {}
    for name, k in bias_specs:
        bcd[name] = nc.dram_tensor(name, [128, k], F32, kind="ExternalInput")
    rv1 = nc.dram_tensor("rv1", [1, D], BF16, kind="ExternalInput")
    rv2 = nc.dram_tensor("rv2", [1, D], BF16, kind="ExternalInput")
    rq1 = nc.dram_tensor("rq1", [1, D], BF16, kind="ExternalInput")
    rk1 = nc.dram_tensor("rk1", [1, D], BF16, kind="ExternalInput")
    rq2 = nc.dram_tensor("rq2", [1, D], BF16, kind="ExternalInput")
    rk2 = nc.dram_tensor("rk2", [1, D], BF16, kind="ExternalInput")
    wseld = nc.dram_tensor("wsel", [4, 256], BF16, kind="ExternalInput")
    bigmask = nc.dram_tensor("bigmask", [2, 8, 128, 512], BF16, kind="ExternalInput")
    outT = nc.dram_tensor("outT", [D, QL], F32, kind="ExternalOutput")
    v2locA = nc.dram_tensor("v2locA", [QL // 2, D], BF16)
    v2locB = nc.dram_tensor("v2locB", [QL // 2, D], BF16)
    v2allA = nc.dram_tensor("v2allA", [QL, D], BF16)
    v2allB = nc.dram_tensor("v2allB", [QL, D], BF16)
    x1spill = nc.dram_tensor("x1spill", [D, QL], F32)
    x2spill = nc.dram_tensor("x2spill", [D, QL], F32)

    def wslices(name):
        return wd[name].ap().rearrange("(mt p) d -> p mt d", p=128)

    xq_d = xTq.ap().rearrange("(mt p) s -> p mt s", p=128)
    x1s_d = x1spill.ap().rearrange("(mt p) s -> p mt s", p=128)
    x2s_d = x2spill.ap().rearrange("(mt p) s -> p mt s", p=128)

    with tile.TileContext(nc) as tc:
        _cms = {}

        def open_pool(**kw):
            cm = tc.tile_pool(**kw)
            _cms[kw["name"]] = cm
            return cm.__enter__()

        def close_pool(pool_name):
            _cms.pop(pool_name).__exit__(None, None, None)

        # long-lived pools (order matters: LIFO close per side)
        const = open_pool(name="const", bufs=1)
        sp1 = open_pool(name="sp1", bufs=1)
        lnsc = open_pool(name="lnsc", bufs=2)
        sump = open_pool(name="sump", bufs=1)
        lnz = open_pool(name="lnz", bufs=1)
        hpool = open_pool(name="hpool", bufs=2)
        epool = open_pool(name="epool", bufs=2)
        usbp = open_pool(name="usbp", bufs=4)
        wpool = open_pool(name="wpool", bufs=2)
        psp = open_pool(name="psp", bufs=1, space="PSUM")

        def ps_s():
            return psp.tile([128, 1024], F32, tag="s", bufs=2, name="ps_s")

        def ps_pp():
            return psp.tile([128, 512], F32, tag="pp", bufs=2, name="ps_pp")

        def ps_u(h):
            return psp.tile([65, 512], F32, tag=f"u{h}", bufs=1, name="ps_u")

        # ---- constants ----
        bc = {}
        for name, k in bias_specs:
            t = const.tile([128, k], F32, tag=name, name=name)
            nc.sync.dma_start(out=t, in_=bcd[name][:, :])
            bc[name] = t
        rv1_sb = const.tile([1, D], BF16, tag="rv1")
        nc.sync.dma_start(out=rv1_sb, in_=rv1[:, :])
        rv2_sb = const.tile([1, D], BF16, tag="rv2")
        nc.sync.dma_start(out=rv2_sb, in_=rv2[:, :])
        rq1_sb = const.tile([1, D], BF16, tag="rq1")
        nc.sync.dma_start(out=rq1_sb, in_=rq1[:, :])
        rk1_sb = const.tile([1, D], BF16, tag="rk1")
        nc.sync.dma_start(out=rk1_sb, in_=rk1[:, :])
        rq2_sb = const.tile([1, D], BF16, tag="rq2")
        nc.sync.dma_start(out=rq2_sb, in_=rq2[:, :])
        rk2_sb = const.tile([1, D], BF16, tag="rk2")
        nc.sync.dma_start(out=rk2_sb, in_=rk2[:, :])
        wsel_sb = const.tile([4, 2, 128], BF16, tag="wsel")
        nc.sync.dma_start(
            out=wsel_sb, in_=wseld.ap().rearrange("p (qb c) -> p qb c", c=128)
        )
        ones_row = const.tile([1, 512], BF16, tag="ones_row")
        nc.vector.memset(ones_row, 1.0)
        ones_col = const.tile([128, 1], BF16, tag="ones_col")
        nc.vector.memset(ones_col, 1.0)
        epsr = const.tile([1, 1], F32, tag="epsr")
        nc.vector.memset(epsr, EPS)

        # ---- filler machinery: A = must finish before next attention,
        #      B = spreadable (fused out-proj) ----
        fillA = deque()
        fillB = deque()

        def _pull(q):
            while q:
                try:
                    next(q[0])
                    return True
                except StopIteration:
                    q.popleft()
            return False

        def drain(n):
            while n > 0:
                if not (_pull(fillA) or _pull(fillB)):
                    break
                n -= 1

        def drain_a():
            while _pull(fillA):
                pass

        def drain_all():
            while _pull(fillA) or _pull(fillB):
                pass

        def ln_block(z, gname, bename, xout, xbfout, qb, xout_qb=None, spill_d=None):
            """LayerNorm over partition(d) axis of z [128, MT, 512] f32."""
            st_ = ps_s()
            psum_s = st_[0:1, 0:512]
            psum_q = st_[0:1, 512:1024]
            for mt in range(MT):
                zbf = lnsc.tile([128, 512], BF16, tag="zbf")
                nc.scalar.activation(zbf[:, :], z[:, mt, :], AF.Copy)
                zsq = lnsc.tile([128, 512], BF16, tag="zsq")
                nc.vector.tensor_mul(zsq[:, :], zbf[:, :], zbf[:, :])
                nc.tensor.matmul(psum_s, ones_col[:, :], zbf[:, :],
                                 start=(mt == 0), stop=(mt == MT - 1))
                nc.tensor.matmul(psum_q, ones_col[:, :], zsq[:, :],
                                 start=(mt == 0), stop=(mt == MT - 1))
                drain(1)
            mu = sp1.tile([1, 512], F32, tag="ln_mu")
            nc.scalar.activation(mu[:, :], psum_s, AF.Copy, scale=1.0 / D)
            t = sp1.tile([1, 512], F32, tag="ln_t")
            nc.scalar.activation(t[:, :], psum_q, AF.Copy, scale=1.0 / D)
            musq = sp1.tile([1, 512], F32, tag="ln_musq")
            nc.vector.tensor_mul(musq[:, :], mu[:, :], mu[:, :])
            nc.vector.tensor_sub(t[:, :], t[:, :], musq[:, :])
            nc.scalar.activation(t[:, :], t[:, :], AF.Sqrt, bias=epsr[:, :])
            rstd = sp1.tile([1, 512], F32, tag="ln_rstd")
            nc.vector.reciprocal(rstd[:, :], t[:, :])
            mubf = sp1.tile([1, 512], BF16, tag="ln_mubf")
            nc.scalar.activation(mubf[:, :], mu[:, :], AF.Copy)
            rsbf = sp1.tile([1, 512], BF16, tag="ln_rsbf")
            nc.scalar.activation(rsbf[:, :], rstd[:, :], AF.Copy)
            mu_b = ps_pp()
            nc.tensor.matmul(mu_b[:, :], ones_row[:, 0:128], mubf[:, :],
                             start=True, stop=True)
            rs_b = ps_pp()
            nc.tensor.matmul(rs_b[:, :], ones_row[:, 0:128], rsbf[:, :],
                             start=True, stop=True)
            qs = slice(qb * 512, qb * 512 + 512)
            oq = qb if xout_qb is None else xout_qb
            oqs = slice(oq * 512, oq * 512 + 512)
            g = bc[gname]
            be = bc[bename]
            for mt in range(MT):
                tmp = lnsc.tile([128, 512], F32, tag="lntmp")
                nc.vector.tensor_sub(tmp[:, :], z[:, mt, :], mu_b[:, :])
                nc.vector.tensor_mul(tmp[:, :], tmp[:, :], rs_b[:, :])
                if xout is None:
                    # stream mode: normalize into a rotating tile, DMA-spill
                    xo = lnsc.tile([128, 512], F32, tag="xo", name="xo")
                    nc.vector.tensor_scalar(
                        xo[:, :], tmp[:, :],
                        g[:, mt:mt + 1], be[:, mt:mt + 1],
                        op0=mybir.AluOpType.mult, op1=mybir.AluOpType.add,
                    )
                    nc.sync.dma_start(out=spill_d[:, mt, qs], in_=xo)
                    if xbfout is not None:
                        nc.scalar.activation(xbfout[:, mt, qs], xo[:, :], AF.Copy)
                else:
                    nc.vector.tensor_scalar(
                        xout[:, mt, oqs], tmp[:, :],
                        g[:, mt:mt + 1], be[:, mt:mt + 1],
                        op0=mybir.AluOpType.mult, op1=mybir.AluOpType.add,
                    )
                    if xbfout is not None:
                        nc.scalar.activation(xbfout[:, mt, qs],
                                             xout[:, mt, oqs], AF.Copy)
                drain(1)

        def attention(KTh, VH, QTh, maskt, attn_h):
            """Merged-head attention for one head pair; output attn_h
            [128, 1024] bf16, softmax-normalized."""
            usbs = {}
            for qb in range(2):
                nkt = SLOT_NKT[qb]
                qs = slice(qb * 512, qb * 512 + 512)
                us = (ps_u(0), ps_u(1))
                prev = None
                for kt in range(nkt):
                    s2 = ps_s()
                    for h in range(2):
                        hs = slice(h * 64, h * 64 + 64)
                        nc.tensor.matmul(
                            s2[:, h * 512:h * 512 + 512],
                            KTh[hs, kt * 128:kt * 128 + 128],
                            QTh[hs, qs],
                            start=True, stop=True,
                        )
                    if prev is not None:
                        e_p, kt_p = prev
                        for h in range(2):
                            nc.tensor.matmul(
                                us[h][:, :], VH[:, kt_p, h, :],
                                e_p[:, h * 512:h * 512 + 512],
                                start=(kt_p == 0), stop=(kt_p == nkt - 1),
                            )
                    e = epool.tile([128, 1024], BF16, tag="e")
                    nc.scalar.activation(e[:, :], s2[:, :], AF.Exp, scale=SCALE)
                    if maskt is not None and kt >= nkt - 8:
                        ki = kt - (nkt - 8)
                        e2 = e[:, :].rearrange("p (two f) -> p two f", two=2)
                        m1 = maskt[:, qb, ki, :].rearrange(
                            "p (one f) -> p one f", one=1)
                        mb, _ = bass.broadcast_tensor_aps(m1, e2)
                        nc.vector.tensor_tensor(out=e2, in0=e2, in1=mb,
                                                op=mybir.AluOpType.mult)
                    prev = (e, kt)
                    drain(2)
                e_p, kt_p = prev
                for h in range(2):
                    nc.tensor.matmul(
                        us[h][:, :], VH[:, kt_p, h, :],
                        e_p[:, h * 512:h * 512 + 512],
                        start=(kt_p == 0), stop=(kt_p == nkt - 1),
                    )
                for h in range(2):
                    usb = usbp.tile([65, 512], F32, tag="usb")
                    nc.vector.tensor_copy(usb[:, :], us[h][:, :])
                    usbs[(qb, h)] = usb
                drain(2)
            sums4 = sump.tile([4, 512], F32, tag="sums4")
            for qb in range(2):
                for h in range(2):
                    r = 2 * qb + h
                    nc.sync.dma_start(
                        out=sums4[r:r + 1, :], in_=usbs[(qb, h)][64:65, :]
                    )
            rec4 = sump.tile([4, 512], F32, tag="rec4")
            nc.vector.reciprocal(rec4[:, :], sums4[:, :])
            recbf4 = sump.tile([4, 512], BF16, tag="recbf4")
            nc.scalar.activation(recbf4[:, :], rec4[:, :], AF.Copy)
            for qb in range(2):
                qs = slice(qb * 512, qb * 512 + 512)
                rb = ps_pp()
                nc.tensor.matmul(rb[:, :], wsel_sb[:, qb, :], recbf4[:, :],
                                 start=True, stop=True)
                for h in range(2):
                    hs = slice(h * 64, h * 64 + 64)
                    nc.vector.tensor_mul(
                        attn_h[hs, qs], usbs[(qb, h)][0:64, :], rb[hs, :]
                    )

        def oproj_ln(attnhs, wname, coname, res_d, res_dt, gname, bename,
                     xbfout, spill_d, wo_pool, after_qb=None):
            """Out-projection via 8-matmul PSUM chains over the 8 saved
            attn head-pair tiles, + bias + residual -> z -> LN -> bf16 out
            (+ f32 half spilled to dram)."""
            zs = {}
            for qb in range(2):
                zs[qb] = lnz.tile([128, MT, 512], F32, tag="z", bufs=2, name="z")
            for nt in range(MT):
                nsl = slice(nt * 128, nt * 128 + 128)
                w = wo_pool.tile([128, MT, 128], BF16, tag="wo", bufs=2, name="wo")
                nc.sync.dma_start(out=w, in_=wslices(wname)[:, :, nsl])
                for qb in range(2):
                    qs = slice(qb * 512, qb * 512 + 512)
                    rt = lnsc.tile([128, 512], res_dt,
                                   tag=f"res{mybir.dt.size(res_dt)}", name="rt")
                    nc.sync.dma_start(out=rt, in_=res_d[:, nt, qs])
                    pp = ps_pp()
                    for dt in range(MT):
                        nc.tensor.matmul(pp[:, :], w[:, dt, :],
                                         attnhs[dt][:, qs],
                                         start=(dt == 0), stop=(dt == MT - 1))
                        if dt % 4 == 3:
                            drain(1)
                    t1 = lnsc.tile([128, 512], F32, tag="lntmp")
                    nc.scalar.activation(t1[:, :], pp[:, :], AF.Identity,
                                         bias=bc[coname][:, nt:nt + 1])
                    nc.vector.tensor_add(zs[qb][:, nt, :], t1[:, :], rt[:, :])
            for qb in range(2):
                ln_block(zs[qb], gname, bename, None, xbfout, qb,
                         spill_d=spill_d)
                if after_qb is not None:
                    after_qb(qb)

        # ================= PHASE A: self-attention =================
        pA = open_pool(name="pA", bufs=1)

        xTs = pA.tile([128, MT, S], BF16, tag="xTs")
        for mt in range(MT):
            nc.sync.dma_start(
                out=xTs[:, mt, :],
                in_=xT.ap().rearrange("(mt p) s -> p mt s", p=128)[:, mt, :],
            )
        xqs = pA.tile([128, MT, QL], BF16, tag="xqs")
        for mt in range(MT):
            nc.sync.dma_start(out=xqs[:, mt, :], in_=xq_d[:, mt, :])
        maskt = pA.tile([128, 2, 8, 512], BF16, tag="maskt")
        nc.sync.dma_start(
            out=maskt, in_=bigmask.ap().rearrange("sl ki p j -> p sl ki j")
        )

        qkv_state = {}

        def gen_qkv(hp):
            ds = slice(hp * 128, hp * 128 + 128)
            wq = wpool.tile([128, MT, 128], BF16, tag="wq", bufs=1, name="wq")
            wk = wpool.tile([128, MT, 128], BF16, tag="wk", bufs=1, name="wk")
            wv = wpool.tile([128, MT, 128], BF16, tag="wv", bufs=1, name="wv")
            for nm, t in (("wq1", wq), ("wk1", wk), ("wv1", wv)):
                nc.sync.dma_start(out=t, in_=wslices(nm)[:, :, ds])
            KTh = hpool.tile([128, S], BF16, tag="KTh", name="KTh")
            QTh = hpool.tile([128, QL], BF16, tag="QTh", name="QTh")
            VH = hpool.tile([128, 16, 2, 65], BF16, tag="VH", name="VH")
            qkv_state[hp] = (KTh, QTh, VH)
            yield
            for sb_ in range(4):
                ss = slice(sb_ * 512, sb_ * 512 + 512)
                pp = ps_pp()
                for mt in range(MT):
                    nc.tensor.matmul(pp[:, :], wk[:, mt, :], xTs[:, mt, ss],
                                     start=(mt == 0), stop=False)
                    if mt % 4 == 3:
                        yield
                nc.tensor.matmul(pp[:, :], rk1_sb[:, ds], ones_row[:, :],
                                 start=False, stop=True)
                nc.vector.tensor_copy(KTh[:, ss], pp[:, :])
            for qb in range(2):
                qs = slice(qb * 512, qb * 512 + 512)
                pp = ps_pp()
                for mt in range(MT):
                    nc.tensor.matmul(pp[:, :], wq[:, mt, :], xqs[:, mt, qs],
                                     start=(mt == 0), stop=False)
                    if mt % 4 == 3:
                        yield
                nc.tensor.matmul(pp[:, :], rq1_sb[:, ds], ones_row[:, :],
                                 start=False, stop=True)
                nc.vector.tensor_copy(QTh[:, qs], pp[:, :])
            nc.vector.memset(VH[:, :, :, 64:65], 1.0)
            for st in range(16):
                ts_ = slice(st * 128, st * 128 + 128)
                pp = ps_pp()
                for mt in range(MT):
                    nc.tensor.matmul(pp[:, 0:128], xTs[:, mt, ts_], wv[:, mt, :],
                                     start=(mt == 0), stop=False)
                nc.tensor.matmul(pp[:, 0:128], ones_row[:, 0:128], rv1_sb[:, ds],
                                 start=False, stop=True)
                nc.vector.tensor_copy(
                    VH[:, st, :, 0:64],
                    pp[:, 0:128].rearrange("p (a b) -> p a b", a=2),
                )
                yield

        fillA.append(gen_qkv(0))
        drain_a()

        attnhs1 = {}
        for hp in range(MT):
            if hp + 1 < MT:
                fillA.append(gen_qkv(hp + 1))
            KTh, QTh, VH = qkv_state.pop(hp)
            attn_h = hpool.tile([128, QL], BF16, tag="attnh", bufs=8)
            attention(KTh, VH, QTh, maskt, attn_h)
            attnhs1[hp] = attn_h
            drain_a()

        close_pool("pA")

        # ---- LN1 (with Q2T projection as filler) -> x1, x1bf ----
        pQ2 = open_pool(name="pQ2", bufs=1)
        Q2T = pQ2.tile([128, MT, QL], BF16, tag="Q2T")
        pENCQ = open_pool(name="pENCQ", bufs=1)
        encq = pENCQ.tile([128, MT, QL], BF16, tag="encq")

        def gen_q2t():
            for mt in range(MT):
                nc.sync.dma_start(
                    out=encq[:, mt, :],
                    in_=encTq.ap().rearrange("(mt p) s -> p mt s", p=128)[:, mt, :],
                )
            yield
            for nt in range(MT):
                nsl = slice(nt * 128, nt * 128 + 128)
                wq2s = wpool.tile([128, MT, 128], BF16, tag="wq", bufs=1, name="wq2s")
                nc.sync.dma_start(out=wq2s, in_=wslices("wq2")[:, :, nsl])
                for qb in range(2):
                    qs = slice(qb * 512, qb * 512 + 512)
                    pp = ps_pp()
                    for mt in range(MT):
                        nc.tensor.matmul(pp[:, :], wq2s[:, mt, :], encq[:, mt, qs],
                                         start=(mt == 0), stop=False)
                        if mt % 4 == 3:
                            yield
                    nc.tensor.matmul(pp[:, :], rq2_sb[:, nsl], ones_row[:, :],
                                     start=False, stop=True)
                    nc.vector.tensor_copy(Q2T[:, nt, qs], pp[:, :])

        fillA.append(gen_q2t())

        pX1B = open_pool(name="pX1B", bufs=1, side="right")
        x1bf = pX1B.tile([128, MT, QL], BF16, tag="x1bf")
        oproj_ln(attnhs1, "wo1", "co1", xq_d, BF16, "g1", "be1",
                 x1bf, x1s_d, wpool)

        drain_all()
        close_pool("pENCQ")

        # ================= V2 projection + split AllGather =================
        pV2 = open_pool(name="pV2", bufs=1)
        v2sb = pV2.tile([128, MT, D], BF16, tag="v2sb")
        for db in range(2):
            dsl = slice(db * 512, db * 512 + 512)
            wv2h = pV2.tile([128, MT, 512], BF16, tag="wv2h", bufs=1, name="wv2h")
            nc.sync.dma_start(out=wv2h, in_=wslices("wv2")[:, :, dsl])
            for st in range(MT):
                ss = slice(st * 128, st * 128 + 128)
                pp = ps_pp()
                for mt in range(MT):
                    nc.tensor.matmul(pp[:, :], x1bf[:, mt, ss], wv2h[:, mt, :],
                                     start=(mt == 0), stop=False)
                nc.tensor.matmul(pp[:, :], ones_row[:, 0:128], rv2_sb[:, dsl],
                                 start=False, stop=True)
                nc.vector.tensor_copy(v2sb[:, st, dsl], pp[:, :])
                drain(2)
                if db == 1 and st == 3:
                    nc.sync.dma_start(
                        out=v2locA.ap().rearrange("(st p) d -> p st d", p=128),
                        in_=v2sb[:, 0:4, :],
                    )
                    nc.gpsimd.collective_compute(
                        "AllGather",
                        mybir.AluOpType.bypass,
                        replica_groups=[[2 * p, 2 * p + 1] for p in range(4)],
                        ins=[v2locA[:, :]],
                        outs=[v2allA[:, :]],
                    )
        nc.sync.dma_start(
            out=v2locB.ap().rearrange("(st p) d -> p st d", p=128),
            in_=v2sb[:, 4:8, :],
        )
        nc.gpsimd.collective_compute(
            "AllGather",
            mybir.AluOpType.bypass,
            replica_groups=[[2 * p, 2 * p + 1] for p in range(4)],
            ins=[v2locB[:, :]],
            outs=[v2allB[:, :]],
        )
        close_pool("pV2")
        close_pool("pX1B")

        # encs + first K2 gens early (filler during V2/AG)
        pENCS = open_pool(name="pENCS", bufs=1)
        encs = pENCS.tile([128, MT, S], BF16, tag="encs")
        for mt in range(MT):
            nc.sync.dma_start(
                out=encs[:, mt, :],
                in_=encT.ap().rearrange("(mt p) s -> p mt s", p=128)[:, mt, :],
            )
        k2_state = {}

        def gen_k2(hp):
            ds = slice(hp * 128, hp * 128 + 128)
            wk2s = wpool.tile([128, MT, 128], BF16, tag="wk", bufs=1, name="wk2s")
            nc.sync.dma_start(out=wk2s, in_=wslices("wk2")[:, :, ds])
            K2h = hpool.tile([128, S], BF16, tag="KTh", name="K2h")
            k2_state[hp] = K2h
            yield
            for sb_ in range(4):
                ss = slice(sb_ * 512, sb_ * 512 + 512)
                pp = ps_pp()
                for mt in range(MT):
                    nc.tensor.matmul(pp[:, :], wk2s[:, mt, :], encs[:, mt, ss],
                                     start=(mt == 0), stop=False)
                    if mt % 4 == 3:
                        yield
                nc.tensor.matmul(pp[:, :], rk2_sb[:, ds], ones_row[:, :],
                                 start=False, stop=True)
                nc.vector.tensor_copy(K2h[:, ss], pp[:, :])

        def vh2_dma(hp):
            VH2 = hpool.tile([128, 16, 2, 65], BF16, tag="VH", name="VH2")
            nc.vector.memset(VH2[:, :, :, 64:65], 1.0)
            # old v2all row r maps to: r<512 -> A[r]; r<1024 -> B[r-512];
            # r<1536 -> A[r-512]; else B[r-1024]
            for t in range(16):
                r = V2_ROW0[t // 4] + (t % 4) * 128
                if r < 512:
                    tens, rn = v2allA, r
                elif r < 1024:
                    tens, rn = v2allB, r - 512
                elif r < 1536:
                    tens, rn = v2allA, r - 512
                else:
                    tens, rn = v2allB, r - 1024
                nc.sync.dma_start(
                    out=VH2[:, t, :, 0:64],
                    in_=tens[rn:rn + 128,
                             hp * 128:hp * 128 + 128].rearrange(
                        "p (a b) -> p a b", a=2),
                )
            return VH2

        fillA.append(gen_k2(0))
        fillA.append(gen_k2(1))
        drain_a()

        vh2_cur = vh2_dma(0)
        k2_queued = 2
        attnhs2 = {}
        for hp in range(MT):
            drain_a()
            if k2_queued < MT:
                fillA.append(gen_k2(k2_queued))
                k2_queued += 1
            K2h = k2_state.pop(hp)
            VH2 = vh2_cur
            if hp + 1 < MT:
                vh2_cur = vh2_dma(hp + 1)
            attn_h = hpool.tile([128, QL], BF16, tag="attnh", bufs=8)
            attention(K2h, VH2, Q2T[:, hp, :], None, attn_h)
            attnhs2[hp] = attn_h

        drain_all()
        close_pool("pENCS")
        close_pool("pQ2")

        # ---- LN2 -> x2bf (full) + x2 spilled to dram ----
        pX2B = open_pool(name="pX2B", bufs=1, side="right")
        x2bf = pX2B.tile([128, MT, QL], BF16, tag="x2bf")
        close_pool("wpool")
        close_pool("usbp")
        close_pool("epool")
        pF2 = open_pool(name="pF2", bufs=2)
        pHT = open_pool(name="pHT", bufs=1, side="right")

        def ffn_wf1(qb):
            qs = slice(qb * 512, qb * 512 + 512)
            hT = pHT.tile([128, FT, 512], BF16, tag="hT", name="hT")
            for ft in range(FT):
                wf1s = pF2.tile([128, MT, 128], BF16, tag="wf1s", name="wf1s")
                nc.sync.dma_start(
                    out=wf1s,
                    in_=wd["wf1"].ap().rearrange("(mt p) f -> p mt f", p=128)[
                        :, :, ft * 128:ft * 128 + 128],
                )
                pp = ps_pp()
                for mt in range(MT):
                    nc.tensor.matmul(pp[:, :], wf1s[:, mt, :], x2bf[:, mt, qs],
                                     start=(mt == 0), stop=(mt == MT - 1))
                nc.scalar.activation(hT[:, ft, :], pp[:, :], AF.Relu,
                                     bias=bc["cf1"][:, ft:ft + 1])
            return hT

        def ffn_wf2(qb, hT):
            qs = slice(qb * 512, qb * 512 + 512)
            z3 = lnz.tile([128, MT, 512], F32, tag="z", bufs=2, name="z3")
            for nt in range(MT):
                wf2s = pF2.tile([128, FT, 128], BF16, tag="wf2s", name="wf2s")
                nc.sync.dma_start(
                    out=wf2s,
                    in_=wd["wf2"].ap().rearrange("(ft p) d -> p ft d", p=128)[
                        :, :, nt * 128:nt * 128 + 128],
                )
                rt = lnsc.tile([128, 512], F32, tag="res4", name="rt")
                nc.sync.dma_start(out=rt, in_=x2s_d[:, nt, qs])
                pp = ps_pp()
                for ft in range(FT):
                    nc.tensor.matmul(pp[:, :], wf2s[:, ft, :], hT[:, ft, :],
                                     start=(ft == 0), stop=(ft == FT - 1))
                t1 = lnsc.tile([128, 512], F32, tag="lntmp")
                nc.scalar.activation(t1[:, :], pp[:, :], AF.Identity,
                                     bias=bc["cf2"][:, nt:nt + 1])
                nc.vector.tensor_add(z3[:, nt, :], t1[:, :], rt[:, :])
            return z3

        out_d = outT.ap().rearrange("(mt p) q -> p mt q", p=128)

        def ffn_ln3(qb, z3):
            ln_block(z3, "g3", "be3", None, None, qb, spill_d=out_d)

        ffn_state = {}

        def _after_ln2(qb):
            if qb == 0:
                ffn_state[0] = ffn_wf1(0)

        oproj_ln(attnhs2, "wo2", "co2", x1s_d, F32, "g2", "be2",
                 x2bf, x2s_d, pF2, after_qb=_after_ln2)

        hT0 = ffn_state[0]
        z30 = ffn_wf2(0, hT0)
        hT1 = ffn_wf1(1)
        ffn_ln3(0, z30)
        z31 = ffn_wf2(1, hT1)
        ffn_ln3(1, z31)
        close_pool("pHT")
        close_pool("pF2")
        close_pool("pX2B")

        for nm in reversed(list(_cms)):
            close_pool(nm)

    return nc


_CACHED = {}


def _get_nc():
    if "nc" not in _CACHED:
        nc = build_nc()
        legalize_waits(nc)
        _CACHED["nc"] = nc
    return _CACHED["nc"]


def _colbias(v, k=8):
    return np.ascontiguousarray(np.asarray(v, np.float32).reshape(k, 128).T)


def _bf(a):
    return np.ascontiguousarray(np.asarray(a)).astype(ml_dtypes.bfloat16)


def _make_mask(j):
    q0s = (0, 1536) if j == 0 else (512, 1024)
    m = np.zeros((2, 8, 128, 512), np.float32)
    for sl in range(2):
        q0 = q0s[sl]
        for ki in range(8):
            kt = ki if sl == 0 else 8 + ki
            k0 = kt * 128
            i = np.arange(128)[:, None]
            jq = np.arange(512)[None, :]
            m[sl, ki] = ((q0 + jq) >= (k0 + i)).astype(np.float32)
    return m.astype(ml_dtypes.bfloat16)


def _make_wsel():
    w = np.zeros((4, 256), np.float32)
    for qb in range(2):
        for r in range(128):
            w[qb * 2 + (r // 64), qb * 128 + r] = 1.0
    return w.astype(ml_dtypes.bfloat16)


def kernel(**inputs):
    x = np.asarray(inputs["x"], np.float32)
    enc = np.asarray(inputs["encoder_output"], np.float32)
    shared = {}
    for name in ("wq1", "wk1", "wv1", "wo1", "wq2", "wk2", "wv2", "wo2",
                 "wf1", "wf2"):
        shared[name] = _bf(inputs[name])
    for src, dst in (("bq1", "cq1"), ("bk1", "ck1"), ("bo1", "co1"),
                     ("bq2", "cq2"), ("bk2", "ck2"), ("bo2", "co2"),
                     ("g1", "g1"), ("be1", "be1"), ("g2", "g2"), ("be2", "be2"),
                     ("g3", "g3"), ("be3", "be3")):
        shared[dst] = _colbias(inputs[src], 8)
    shared["cf1"] = _colbias(inputs["bf1"], 32)
    shared["cf2"] = _colbias(inputs["bf2"], 8)
    shared["rv1"] = _bf(np.asarray(inputs["bv1"]).reshape(1, D))
    shared["rv2"] = _bf(np.asarray(inputs["bv2"]).reshape(1, D))
    shared["rq1"] = _bf(np.asarray(inputs["bq1"]).reshape(1, D))
    shared["rk1"] = _bf(np.asarray(inputs["bk1"]).reshape(1, D))
    shared["rq2"] = _bf(np.asarray(inputs["bq2"]).reshape(1, D))
    shared["rk2"] = _bf(np.asarray(inputs["bk2"]).reshape(1, D))
    shared["wsel"] = _make_wsel()
    masks = {0: _make_mask(0), 1: _make_mask(1)}

    in_maps = []
    col_list = []
    for c in range(NCORES):
        b, j = c // 2, c % 2
        q0a, q0b = (0, 1536) if j == 0 else (512, 1024)
        cols = np.r_[q0a:q0a + 512, q0b:q0b + 512]
        col_list.append((b, cols))
        xTb = np.ascontiguousarray(x[b].T)
        encTb = np.ascontiguousarray(enc[b].T)
        m = dict(shared)
        m["xT"] = _bf(xTb)
        m["xTq"] = _bf(xTb[:, cols])
        m["encT"] = _bf(encTb)
        m["encTq"] = _bf(encTb[:, cols])
        m["bigmask"] = masks[j]
        in_maps.append(m)

    global _LAST_IN_MAPS
    _LAST_IN_MAPS = in_maps
    nc = _get_nc()
    res = run_bass_kernel_spmd(nc, in_maps, core_ids=list(range(NCORES)))
    out = np.empty((B, S, D), np.float32)
    for c in range(NCORES):
        b, cols = col_list[c]
        out[b, cols, :] = res.results[c]["outT"].T
    return out


# revision 6
# speedup vs baseline: 1.0193x; 1.0193x over previous
"""Trainium2 Bass kernel for a post-LN transformer decoder layer (v2).

Sharding: 8 cores = 4 batches x 2 token-halves. Transposed layout
[d_model, tokens]; matmul operands bf16 (fp32 PSUM accumulate).

v2 restructure for PE p-state + engine overlap:
- Merged [128,1024] score PSUM tile per k-tile (both heads), ONE exp per
  k-tile, software-pipelined PV (scores of kt issued before PV of kt-1).
- Two filler queues interleave independent projection matmuls into the
  ACT-bound attention stream so the tensor engine never idles.
- Out-proj fused: attn(hp) is consumed immediately by 16 per-hp out-proj
  matmuls accumulated into an SBUF f32 tile; doubles as attention filler.
- Softmax normalize: per-hp batched reciprocal ([4,512] via DMA row
  gather; reciprocal cost is per-free-element, partitions are free).
- Residuals streamed from DRAM at z-assembly time; x1 spilled to DRAM
  across the cross-attention phase (SBUF pressure).
"""

import sys

sys.path.insert(0, "/opt/trn_rl_repo")

from collections import deque

import numpy as np
import ml_dtypes

import concourse.bass as bass
import concourse.tile as tile
from concourse import mybir
from concourse.bass_utils import run_bass_kernel_spmd

BF16 = mybir.dt.bfloat16
F32 = mybir.dt.float32
AF = mybir.ActivationFunctionType

D = 1024
DFF = 4096
B, S = 4, 2048
NCORES = 8
QL = 1024
EPS = 1e-6
MT = 8
FT = 32
SCALE = 0.125
SLOT_NKT = (8, 16)

V2_ROW0 = [0, 1024, 1536, 512]


def legalize_waits(nc, max_waits=1):
    nid = 0
    for fn in nc.m.functions:
        for bb in fn.blocks:
            new = []
            changed = False
            for inst in bb.instructions:
                si = inst.sync_info
                if si is not None and si.on_wait and len(si.on_wait) > max_waits:
                    waits = list(si.on_wait)
                    for w in waits[:-max_waits]:
                        nid += 1
                        nop = mybir.InstNoOp(name=f"I-waitfix-{nid}", ins=[], outs=[])
                        nop.engine = inst.engine
                        nop.sync_info = mybir.SyncInfo(on_wait=[w], on_update=[])
                        new.append(nop)
                    inst.sync_info = mybir.SyncInfo(
                        on_wait=waits[-max_waits:], on_update=list(si.on_update)
                    )
                    changed = True
                new.append(inst)
            if changed:
                bb.instructions = new


def build_nc():
    nc = bass.Bass(num_devices=NCORES)

    xT = nc.dram_tensor("xT", [D, S], BF16, kind="ExternalInput")
    xTq = nc.dram_tensor("xTq", [D, QL], BF16, kind="ExternalInput")
    encT = nc.dram_tensor("encT", [D, S], BF16, kind="ExternalInput")
    encTq = nc.dram_tensor("encTq", [D, QL], BF16, kind="ExternalInput")
    wd = {}
    for name in ("wq1", "wk1", "wv1", "wo1", "wq2", "wk2", "wv2", "wo2"):
        wd[name] = nc.dram_tensor(name, [D, D], BF16, kind="ExternalInput")
    wd["wf1"] = nc.dram_tensor("wf1", [D, DFF], BF16, kind="ExternalInput")
    wd["wf2"] = nc.dram_tensor("wf2", [DFF, D], BF16, kind="ExternalInput")
    bias_specs = (
        ("cq1", 8), ("ck1", 8), ("co1", 8), ("cq2", 8), ("ck2", 8), ("co2", 8),
        ("cf1", 32), ("cf2", 8),
        ("g1", 8), ("be1", 8), ("g2", 8), ("be2", 8), ("g3", 8), ("be3", 8),
    )
    bcd = {}
    for name, k in bias_specs:
        bcd[name] = nc.dram_tensor(name, [128, k], F32, kind="ExternalInput")
    rv1 = nc.dram_tensor("rv1", [1, D], BF16, kind="ExternalInput")
    rv2 = nc.dram_tensor("rv2", [1, D], BF16, kind="ExternalInput")
    rq1 = nc.dram_tensor("rq1", [1, D], BF16, kind="ExternalInput")
    rk1 = nc.dram_tensor("rk1", [1, D], BF16, kind="ExternalInput")
    rq2 = nc.dram_tensor("rq2", [1, D], BF16, kind="ExternalInput")
    rk2 = nc.dram_tensor("rk2", [1, D], BF16, kind="ExternalInput")
    wseld = nc.dram_tensor("wsel", [4, 256], BF16, kind="ExternalInput")
    bigmask = nc.dram_tensor("bigmask", [2, 8, 128, 512], BF16, kind="ExternalInput")
    outT = nc.dram_tensor("outT", [D, QL], F32, kind="ExternalOutput")
    v2locA = nc.dram_tensor("v2locA", [QL // 2, D], BF16)
    v2locB = nc.dram_tensor("v2locB", [QL // 2, D], BF16)
    v2allA = nc.dram_tensor("v2allA", [QL, D], BF16)
    v2allB = nc.dram_tensor("v2allB", [QL, D], BF16)
    x1spill = nc.dram_tensor("x1spill", [D, QL], F32)
    x2spill = nc.dram_tensor("x2spill", [D, QL], F32)

    def wslices(name):
        return wd[name].ap().rearrange("(mt p) d -> p mt d", p=128)

    xq_d = xTq.ap().rearrange("(mt p) s -> p mt s", p=128)
    x1s_d = x1spill.ap().rearrange("(mt p) s -> p mt s", p=128)
    x2s_d = x2spill.ap().rearrange("(mt p) s -> p mt s", p=128)

    with tile.TileContext(nc) as tc:
        _cms = {}

        def open_pool(**kw):
            cm = tc.tile_pool(**kw)
            _cms[kw["name"]] = cm
            return cm.__enter__()

        def close_pool(pool_name):
            _cms.pop(pool_name).__exit__(None, None, None)

        # long-lived pools (order matters: LIFO close per side)
        const = open_pool(name="const", bufs=1)
        sp1 = open_pool(name="sp1", bufs=1)
        lnsc = open_pool(name="lnsc", bufs=2)
        sump = open_pool(name="sump", bufs=1)
        lnz = open_pool(name="lnz", bufs=1)
        hpool = open_pool(name="hpool", bufs=2)
        epool = open_pool(name="epool", bufs=2)
        usbp = open_pool(name="usbp", bufs=4)
        wpool = open_pool(name="wpool", bufs=2)
        psp = open_pool(name="psp", bufs=1, space="PSUM")

        def ps_s():
            return psp.tile([128, 1024], F32, tag="s", bufs=2, name="ps_s")

        def ps_pp():
            return psp.tile([128, 512], F32, tag="pp", bufs=2, name="ps_pp")

        def ps_u(h):
            return psp.tile([65, 512], F32, tag=f"u{h}", bufs=1, name="ps_u")

        # ---- constants ----
        bc = {}
        for name, k in bias_specs:
            t = const.tile([128, k], F32, tag=name, name=name)
            nc.sync.dma_start(out=t, in_=bcd[name][:, :])
            bc[name] = t
        rv1_sb = const.tile([1, D], BF16, tag="rv1")
        nc.sync.dma_start(out=rv1_sb, in_=rv1[:, :])
        rv2_sb = const.tile([1, D], BF16, tag="rv2")
        nc.sync.dma_start(out=rv2_sb, in_=rv2[:, :])
        rq1_sb = const.tile([1, D], BF16, tag="rq1")
        nc.sync.dma_start(out=rq1_sb, in_=rq1[:, :])
        rk1_sb = const.tile([1, D], BF16, tag="rk1")
        nc.sync.dma_start(out=rk1_sb, in_=rk1[:, :])
        rq2_sb = const.tile([1, D], BF16, tag="rq2")
        nc.sync.dma_start(out=rq2_sb, in_=rq2[:, :])
        rk2_sb = const.tile([1, D], BF16, tag="rk2")
        nc.sync.dma_start(out=rk2_sb, in_=rk2[:, :])
        wsel_sb = const.tile([4, 2, 128], BF16, tag="wsel")
        nc.sync.dma_start(
            out=wsel_sb, in_=wseld.ap().rearrange("p (qb c) -> p qb c", c=128)
        )
        ones_row = const.tile([1, 512], BF16, tag="ones_row")
        nc.vector.memset(ones_row, 1.0)
        ones_col = const.tile([128, 1], BF16, tag="ones_col")
        nc.vector.memset(ones_col, 1.0)
        epsr = const.tile([1, 1], F32, tag="epsr")
        nc.vector.memset(epsr, EPS)

        # ---- filler machinery: A = must finish before next attention,
        #      B = spreadable (fused out-proj) ----
        fillA = deque()
        fillB = deque()

        def _pull(q):
            while q:
                try:
                    next(q[0])
                    return True
                except StopIteration:
                    q.popleft()
            return False

        def drain(n):
            while n > 0:
                if not (_pull(fillA) or _pull(fillB)):
                    break
                n -= 1

        def drain_a():
            while _pull(fillA):
                pass

        def drain_all():
            while _pull(fillA) or _pull(fillB):
                pass

        def ln_block(z, gname, bename, xout, xbfout, qb, xout_qb=None, spill_d=None):
            """LayerNorm over partition(d) axis of z [128, MT, 512] f32."""
            st_ = ps_s()
            psum_s = st_[0:1, 0:512]
            psum_q = st_[0:1, 512:1024]
            for mt in range(MT):
                zbf = lnsc.tile([128, 512], BF16, tag="zbf")
                nc.scalar.activation(zbf[:, :], z[:, mt, :], AF.Copy)
                zsq = lnsc.tile([128, 512], BF16, tag="zsq")
                nc.vector.tensor_mul(zsq[:, :], zbf[:, :], zbf[:, :])
                nc.tensor.matmul(psum_s, ones_col[:, :], zbf[:, :],
                                 start=(mt == 0), stop=(mt == MT - 1))
                nc.tensor.matmul(psum_q, ones_col[:, :], zsq[:, :],
                                 start=(mt == 0), stop=(mt == MT - 1))
                drain(1)
            mu = sp1.tile([1, 512], F32, tag="ln_mu")
            nc.scalar.activation(mu[:, :], psum_s, AF.Copy, scale=1.0 / D)
            t = sp1.tile([1, 512], F32, tag="ln_t")
            nc.scalar.activation(t[:, :], psum_q, AF.Copy, scale=1.0 / D)
            musq = sp1.tile([1, 512], F32, tag="ln_musq")
            nc.vector.tensor_mul(musq[:, :], mu[:, :], mu[:, :])
            nc.vector.tensor_sub(t[:, :], t[:, :], musq[:, :])
            nc.scalar.activation(t[:, :], t[:, :], AF.Sqrt, bias=epsr[:, :])
            rstd = sp1.tile([1, 512], F32, tag="ln_rstd")
            nc.vector.reciprocal(rstd[:, :], t[:, :])
            mubf = sp1.tile([1, 512], BF16, tag="ln_mubf")
            nc.scalar.activation(mubf[:, :], mu[:, :], AF.Copy)
            rsbf = sp1.tile([1, 512], BF16, tag="ln_rsbf")
            nc.scalar.activation(rsbf[:, :], rstd[:, :], AF.Copy)
            mu_b = ps_pp()
            nc.tensor.matmul(mu_b[:, :], ones_row[:, 0:128], mubf[:, :],
                             start=True, stop=True)
            rs_b = ps_pp()
            nc.tensor.matmul(rs_b[:, :], ones_row[:, 0:128], rsbf[:, :],
                             start=True, stop=True)
            qs = slice(qb * 512, qb * 512 + 512)
            oq = qb if xout_qb is None else xout_qb
            oqs = slice(oq * 512, oq * 512 + 512)
            g = bc[gname]
            be = bc[bename]
            for mt in range(MT):
                tmp = lnsc.tile([128, 512], F32, tag="lntmp")
                nc.vector.tensor_sub(tmp[:, :], z[:, mt, :], mu_b[:, :])
                nc.vector.tensor_mul(tmp[:, :], tmp[:, :], rs_b[:, :])
                if xout is None:
                    # stream mode: normalize into a rotating tile, DMA-spill
                    xo = lnsc.tile([128, 512], F32, tag="xo", name="xo")
                    nc.vector.tensor_scalar(
                        xo[:, :], tmp[:, :],
                        g[:, mt:mt + 1], be[:, mt:mt + 1],
                        op0=mybir.AluOpType.mult, op1=mybir.AluOpType.add,
                    )
                    nc.sync.dma_start(out=spill_d[:, mt, qs], in_=xo)
                    if xbfout is not None:
                        nc.scalar.activation(xbfout[:, mt, qs], xo[:, :], AF.Copy)
                else:
                    nc.vector.tensor_scalar(
                        xout[:, mt, oqs], tmp[:, :],
                        g[:, mt:mt + 1], be[:, mt:mt + 1],
                        op0=mybir.AluOpType.mult, op1=mybir.AluOpType.add,
                    )
                    if xbfout is not None:
                        nc.scalar.activation(xbfout[:, mt, qs],
                                             xout[:, mt, oqs], AF.Copy)
                drain(1)

        def attention(KTh, VH, QTh, maskt, attn_h):
            """Merged-head attention for one head pair; output attn_h
            [128, 1024] bf16, softmax-normalized."""
            usbs = {}
            for qb in range(2):
                nkt = SLOT_NKT[qb]
                qs = slice(qb * 512, qb * 512 + 512)
                us = (ps_u(0), ps_u(1))
                prev = None
                for kt in range(nkt):
                    s2 = ps_s()
                    for h in range(2):
                        hs = slice(h * 64, h * 64 + 64)
                        nc.tensor.matmul(
                            s2[:, h * 512:h * 512 + 512],
                            KTh[hs, kt * 128:kt * 128 + 128],
                            QTh[hs, qs],
                            start=True, stop=True,
                        )
                    if prev is not None:
                        e_p, kt_p = prev
                        for h in range(2):
                            nc.tensor.matmul(
                                us[h][:, :], VH[:, kt_p, h, :],
                                e_p[:, h * 512:h * 512 + 512],
                                start=(kt_p == 0), stop=(kt_p == nkt - 1),
                            )
                    e = epool.tile([128, 1024], BF16, tag="e")
                    nc.scalar.activation(e[:, :], s2[:, :], AF.Exp, scale=SCALE)
                    if maskt is not None and kt >= nkt - 8:
                        ki = kt - (nkt - 8)
                        e2 = e[:, :].rearrange("p (two f) -> p two f", two=2)
                        m1 = maskt[:, qb, ki, :].rearrange(
                            "p (one f) -> p one f", one=1)
                        mb, _ = bass.broadcast_tensor_aps(m1, e2)
                        nc.vector.tensor_tensor(out=e2, in0=e2, in1=mb,
                                                op=mybir.AluOpType.mult)
                    prev = (e, kt)
                    drain(2)
                e_p, kt_p = prev
                for h in range(2):
                    nc.tensor.matmul(
                        us[h][:, :], VH[:, kt_p, h, :],
                        e_p[:, h * 512:h * 512 + 512],
                        start=(kt_p == 0), stop=(kt_p == nkt - 1),
                    )
                for h in range(2):
                    usb = usbp.tile([65, 512], F32, tag="usb")
                    nc.vector.tensor_copy(usb[:, :], us[h][:, :])
                    usbs[(qb, h)] = usb
                drain(2)
            sums4 = sump.tile([4, 512], F32, tag="sums4")
            for qb in range(2):
                for h in range(2):
                    r = 2 * qb + h
                    nc.sync.dma_start(
                        out=sums4[r:r + 1, :], in_=usbs[(qb, h)][64:65, :]
                    )
            rec4 = sump.tile([4, 512], F32, tag="rec4")
            nc.vector.reciprocal(rec4[:, :], sums4[:, :])
            recbf4 = sump.tile([4, 512], BF16, tag="recbf4")
            nc.scalar.activation(recbf4[:, :], rec4[:, :], AF.Copy)
            for qb in range(2):
                qs = slice(qb * 512, qb * 512 + 512)
                rb = ps_pp()
                nc.tensor.matmul(rb[:, :], wsel_sb[:, qb, :], recbf4[:, :],
                                 start=True, stop=True)
                for h in range(2):
                    hs = slice(h * 64, h * 64 + 64)
                    nc.vector.tensor_mul(
                        attn_h[hs, qs], usbs[(qb, h)][0:64, :], rb[hs, :]
                    )

        def oproj_ln(attnhs, wname, coname, res_d, res_dt, gname, bename,
                     xbfout, spill_d):
            """Out-projection via 8-matmul PSUM chains over the 8 saved
            attn head-pair tiles, + bias + residual -> z -> LN -> bf16 out
            (+ f32 half spilled to dram)."""
            zs = {}
            for qb in range(2):
                zs[qb] = lnz.tile([128, MT, 512], F32, tag="z", bufs=2, name="z")
            for nt in range(MT):
                nsl = slice(nt * 128, nt * 128 + 128)
                w = wpool.tile([128, MT, 128], BF16, tag="wo", bufs=2, name="wo")
                nc.sync.dma_start(out=w, in_=wslices(wname)[:, :, nsl])
                for qb in range(2):
                    qs = slice(qb * 512, qb * 512 + 512)
                    rt = lnsc.tile([128, 512], res_dt,
                                   tag=f"res{mybir.dt.size(res_dt)}", name="rt")
                    nc.sync.dma_start(out=rt, in_=res_d[:, nt, qs])
                    pp = ps_pp()
                    for dt in range(MT):
                        nc.tensor.matmul(pp[:, :], w[:, dt, :],
                                         attnhs[dt][:, qs],
                                         start=(dt == 0), stop=(dt == MT - 1))
                        if dt % 4 == 3:
                            drain(1)
                    t1 = lnsc.tile([128, 512], F32, tag="lntmp")
                    nc.scalar.activation(t1[:, :], pp[:, :], AF.Identity,
                                         bias=bc[coname][:, nt:nt + 1])
                    nc.vector.tensor_add(zs[qb][:, nt, :], t1[:, :], rt[:, :])
            for qb in range(2):
                ln_block(zs[qb], gname, bename, None, xbfout, qb,
                         spill_d=spill_d)

        # ================= PHASE A: self-attention =================
        pA = open_pool(name="pA", bufs=1)

        xTs = pA.tile([128, MT, S], BF16, tag="xTs")
        for mt in range(MT):
            nc.sync.dma_start(
                out=xTs[:, mt, :],
                in_=xT.ap().rearrange("(mt p) s -> p mt s", p=128)[:, mt, :],
            )
        xqs = pA.tile([128, MT, QL], BF16, tag="xqs")
        for mt in range(MT):
            nc.sync.dma_start(out=xqs[:, mt, :], in_=xq_d[:, mt, :])
        maskt = pA.tile([128, 2, 8, 512], BF16, tag="maskt")
        nc.sync.dma_start(
            out=maskt, in_=bigmask.ap().rearrange("sl ki p j -> p sl ki j")
        )

        qkv_state = {}

        def gen_qkv(hp):
            ds = slice(hp * 128, hp * 128 + 128)
            wq = wpool.tile([128, MT, 128], BF16, tag="wq", bufs=1, name="wq")
            wk = wpool.tile([128, MT, 128], BF16, tag="wk", bufs=1, name="wk")
            wv = wpool.tile([128, MT, 128], BF16, tag="wv", bufs=1, name="wv")
            for nm, t in (("wq1", wq), ("wk1", wk), ("wv1", wv)):
                nc.sync.dma_start(out=t, in_=wslices(nm)[:, :, ds])
            KTh = hpool.tile([128, S], BF16, tag="KTh", name="KTh")
            QTh = hpool.tile([128, QL], BF16, tag="QTh", name="QTh")
            VH = hpool.tile([128, 16, 2, 65], BF16, tag="VH", name="VH")
            qkv_state[hp] = (KTh, QTh, VH)
            yield
            for sb_ in range(4):
                ss = slice(sb_ * 512, sb_ * 512 + 512)
                pp = ps_pp()
                for mt in range(MT):
                    nc.tensor.matmul(pp[:, :], wk[:, mt, :], xTs[:, mt, ss],
                                     start=(mt == 0), stop=False)
                    if mt % 4 == 3:
                        yield
                nc.tensor.matmul(pp[:, :], rk1_sb[:, ds], ones_row[:, :],
                                 start=False, stop=True)
                nc.vector.tensor_copy(KTh[:, ss], pp[:, :])
            for qb in range(2):
                qs = slice(qb * 512, qb * 512 + 512)
                pp = ps_pp()
                for mt in range(MT):
                    nc.tensor.matmul(pp[:, :], wq[:, mt, :], xqs[:, mt, qs],
                                     start=(mt == 0), stop=False)
                    if mt % 4 == 3:
                        yield
                nc.tensor.matmul(pp[:, :], rq1_sb[:, ds], ones_row[:, :],
                                 start=False, stop=True)
                nc.vector.tensor_copy(QTh[:, qs], pp[:, :])
            nc.vector.memset(VH[:, :, :, 64:65], 1.0)
            for st in range(16):
                ts_ = slice(st * 128, st * 128 + 128)
                pp = ps_pp()
                for mt in range(MT):
                    nc.tensor.matmul(pp[:, 0:128], xTs[:, mt, ts_], wv[:, mt, :],
                                     start=(mt == 0), stop=False)
                nc.tensor.matmul(pp[:, 0:128], ones_row[:, 0:128], rv1_sb[:, ds],
                                 start=False, stop=True)
                nc.vector.tensor_copy(
                    VH[:, st, :, 0:64],
                    pp[:, 0:128].rearrange("p (a b) -> p a b", a=2),
                )
                yield

        fillA.append(gen_qkv(0))
        drain_a()

        attnhs1 = {}
        for hp in range(MT):
            if hp + 1 < MT:
                fillA.append(gen_qkv(hp + 1))
            KTh, QTh, VH = qkv_state.pop(hp)
            attn_h = hpool.tile([128, QL], BF16, tag="attnh", bufs=8)
            attention(KTh, VH, QTh, maskt, attn_h)
            attnhs1[hp] = attn_h
            drain_a()

        close_pool("pA")

        # ---- LN1 (with Q2T projection as filler) -> x1, x1bf ----
        pQ2 = open_pool(name="pQ2", bufs=1)
        Q2T = pQ2.tile([128, MT, QL], BF16, tag="Q2T")
        pENCQ = open_pool(name="pENCQ", bufs=1)
        encq = pENCQ.tile([128, MT, QL], BF16, tag="encq")

        def gen_q2t():
            for mt in range(MT):
                nc.sync.dma_start(
                    out=encq[:, mt, :],
                    in_=encTq.ap().rearrange("(mt p) s -> p mt s", p=128)[:, mt, :],
                )
            yield
            for nt in range(MT):
                nsl = slice(nt * 128, nt * 128 + 128)
                wq2s = wpool.tile([128, MT, 128], BF16, tag="wq", bufs=1, name="wq2s")
                nc.sync.dma_start(out=wq2s, in_=wslices("wq2")[:, :, nsl])
                for qb in range(2):
                    qs = slice(qb * 512, qb * 512 + 512)
                    pp = ps_pp()
                    for mt in range(MT):
                        nc.tensor.matmul(pp[:, :], wq2s[:, mt, :], encq[:, mt, qs],
                                         start=(mt == 0), stop=False)
                        if mt % 4 == 3:
                            yield
                    nc.tensor.matmul(pp[:, :], rq2_sb[:, nsl], ones_row[:, :],
                                     start=False, stop=True)
                    nc.vector.tensor_copy(Q2T[:, nt, qs], pp[:, :])

        fillA.append(gen_q2t())

        pX1B = open_pool(name="pX1B", bufs=1, side="right")
        x1bf = pX1B.tile([128, MT, QL], BF16, tag="x1bf")
        oproj_ln(attnhs1, "wo1", "co1", xq_d, BF16, "g1", "be1",
                 x1bf, x1s_d)

        drain_all()
        close_pool("pENCQ")

        # ================= V2 projection + split AllGather =================
        pV2 = open_pool(name="pV2", bufs=1)
        v2sb = pV2.tile([128, MT, D], BF16, tag="v2sb")
        for db in range(2):
            dsl = slice(db * 512, db * 512 + 512)
            wv2h = pV2.tile([128, MT, 512], BF16, tag="wv2h", bufs=1, name="wv2h")
            nc.sync.dma_start(out=wv2h, in_=wslices("wv2")[:, :, dsl])
            for st in range(MT):
                ss = slice(st * 128, st * 128 + 128)
                pp = ps_pp()
                for mt in range(MT):
                    nc.tensor.matmul(pp[:, :], x1bf[:, mt, ss], wv2h[:, mt, :],
                                     start=(mt == 0), stop=False)
                nc.tensor.matmul(pp[:, :], ones_row[:, 0:128], rv2_sb[:, dsl],
                                 start=False, stop=True)
                nc.vector.tensor_copy(v2sb[:, st, dsl], pp[:, :])
                drain(2)
                if db == 1 and st == 3:
                    nc.sync.dma_start(
                        out=v2locA.ap().rearrange("(st p) d -> p st d", p=128),
                        in_=v2sb[:, 0:4, :],
                    )
                    nc.gpsimd.collective_compute(
                        "AllGather",
                        mybir.AluOpType.bypass,
                        replica_groups=[[2 * p, 2 * p + 1] for p in range(4)],
                        ins=[v2locA[:, :]],
                        outs=[v2allA[:, :]],
                    )
        nc.sync.dma_start(
            out=v2locB.ap().rearrange("(st p) d -> p st d", p=128),
            in_=v2sb[:, 4:8, :],
        )
        nc.gpsimd.collective_compute(
            "AllGather",
            mybir.AluOpType.bypass,
            replica_groups=[[2 * p, 2 * p + 1] for p in range(4)],
            ins=[v2locB[:, :]],
            outs=[v2allB[:, :]],
        )
        close_pool("pV2")
        close_pool("pX1B")

        # encs + first K2 gens early (filler during V2/AG)
        pENCS = open_pool(name="pENCS", bufs=1)
        encs = pENCS.tile([128, MT, S], BF16, tag="encs")
        for mt in range(MT):
            nc.sync.dma_start(
                out=encs[:, mt, :],
                in_=encT.ap().rearrange("(mt p) s -> p mt s", p=128)[:, mt, :],
            )
        k2_state = {}

        def gen_k2(hp):
            ds = slice(hp * 128, hp * 128 + 128)
            wk2s = wpool.tile([128, MT, 128], BF16, tag="wk", bufs=1, name="wk2s")
            nc.sync.dma_start(out=wk2s, in_=wslices("wk2")[:, :, ds])
            K2h = hpool.tile([128, S], BF16, tag="KTh", name="K2h")
            k2_state[hp] = K2h
            yield
            for sb_ in range(4):
                ss = slice(sb_ * 512, sb_ * 512 + 512)
                pp = ps_pp()
                for mt in range(MT):
                    nc.tensor.matmul(pp[:, :], wk2s[:, mt, :], encs[:, mt, ss],
                                     start=(mt == 0), stop=False)
                    if mt % 4 == 3:
                        yield
                nc.tensor.matmul(pp[:, :], rk2_sb[:, ds], ones_row[:, :],
                                 start=False, stop=True)
                nc.vector.tensor_copy(K2h[:, ss], pp[:, :])

        def vh2_dma(hp):
            VH2 = hpool.tile([128, 16, 2, 65], BF16, tag="VH", name="VH2")
            nc.vector.memset(VH2[:, :, :, 64:65], 1.0)
            # old v2all row r maps to: r<512 -> A[r]; r<1024 -> B[r-512];
            # r<1536 -> A[r-512]; else B[r-1024]
            for t in range(16):
                r = V2_ROW0[t // 4] + (t % 4) * 128
                if r < 512:
                    tens, rn = v2allA, r
                elif r < 1024:
                    tens, rn = v2allB, r - 512
                elif r < 1536:
                    tens, rn = v2allA, r - 512
                else:
                    tens, rn = v2allB, r - 1024
                nc.sync.dma_start(
                    out=VH2[:, t, :, 0:64],
                    in_=tens[rn:rn + 128,
                             hp * 128:hp * 128 + 128].rearrange(
                        "p (a b) -> p a b", a=2),
                )
            return VH2

        fillA.append(gen_k2(0))
        fillA.append(gen_k2(1))
        drain_a()

        vh2_cur = vh2_dma(0)
        k2_queued = 2
        attnhs2 = {}
        for hp in range(MT):
            drain_a()
            if k2_queued < MT:
                fillA.append(gen_k2(k2_queued))
                k2_queued += 1
            K2h = k2_state.pop(hp)
            VH2 = vh2_cur
            if hp + 1 < MT:
                vh2_cur = vh2_dma(hp + 1)
            attn_h = hpool.tile([128, QL], BF16, tag="attnh", bufs=8)
            attention(K2h, VH2, Q2T[:, hp, :], None, attn_h)
            attnhs2[hp] = attn_h

        drain_all()
        close_pool("pENCS")
        close_pool("pQ2")

        # ---- LN2 -> x2bf (full) + x2 spilled to dram ----
        pX2B = open_pool(name="pX2B", bufs=1, side="right")
        x2bf = pX2B.tile([128, MT, QL], BF16, tag="x2bf")
        oproj_ln(attnhs2, "wo2", "co2", x1s_d, F32, "g2", "be2",
                 x2bf, x2s_d)

        # ================= PHASE C: FFN + LN3 =================
        close_pool("wpool")
        close_pool("usbp")
        close_pool("epool")
        close_pool("hpool")
        pF2 = open_pool(name="pF2", bufs=2)
        pHT = open_pool(name="pHT", bufs=2, side="right")

        def ffn_wf1(qb):
            qs = slice(qb * 512, qb * 512 + 512)
            hT = pHT.tile([128, FT, 512], BF16, tag="hT", name="hT")
            for ft in range(FT):
                wf1s = pF2.tile([128, MT, 128], BF16, tag="wf1s", name="wf1s")
                nc.sync.dma_start(
                    out=wf1s,
                    in_=wd["wf1"].ap().rearrange("(mt p) f -> p mt f", p=128)[
                        :, :, ft * 128:ft * 128 + 128],
                )
                pp = ps_pp()
                for mt in range(MT):
                    nc.tensor.matmul(pp[:, :], wf1s[:, mt, :], x2bf[:, mt, qs],
                                     start=(mt == 0), stop=(mt == MT - 1))
                nc.scalar.activation(hT[:, ft, :], pp[:, :], AF.Relu,
                                     bias=bc["cf1"][:, ft:ft + 1])
            return hT

        def ffn_wf2(qb, hT):
            qs = slice(qb * 512, qb * 512 + 512)
            z3 = lnz.tile([128, MT, 512], F32, tag="z", bufs=2, name="z3")
            for nt in range(MT):
                wf2s = pF2.tile([128, FT, 128], BF16, tag="wf2s", bufs=3, name="wf2s")
                nc.sync.dma_start(
                    out=wf2s,
                    in_=wd["wf2"].ap().rearrange("(ft p) d -> p ft d", p=128)[
                        :, :, nt * 128:nt * 128 + 128],
                )
                rt = lnsc.tile([128, 512], F32, tag="res4", name="rt")
                nc.sync.dma_start(out=rt, in_=x2s_d[:, nt, qs])
                pp = ps_pp()
                for ft in range(FT):
                    nc.tensor.matmul(pp[:, :], wf2s[:, ft, :], hT[:, ft, :],
                                     start=(ft == 0), stop=(ft == FT - 1))
                t1 = lnsc.tile([128, 512], F32, tag="lntmp")
                nc.scalar.activation(t1[:, :], pp[:, :], AF.Identity,
                                     bias=bc["cf2"][:, nt:nt + 1])
                nc.vector.tensor_add(z3[:, nt, :], t1[:, :], rt[:, :])
            return z3

        out_d = outT.ap().rearrange("(mt p) q -> p mt q", p=128)

        def ffn_ln3(qb, z3):
            ln_block(z3, "g3", "be3", None, None, qb, spill_d=out_d)

        hT0 = ffn_wf1(0)
        z30 = ffn_wf2(0, hT0)
        hT1 = ffn_wf1(1)
        ffn_ln3(0, z30)
        z31 = ffn_wf2(1, hT1)
        ffn_ln3(1, z31)
        close_pool("pHT")
        close_pool("pF2")
        close_pool("pX2B")

        for nm in reversed(list(_cms)):
            close_pool(nm)

    return nc


_CACHED = {}


def _get_nc():
    if "nc" not in _CACHED:
        nc = build_nc()
        legalize_waits(nc)
        _CACHED["nc"] = nc
    return _CACHED["nc"]


def _colbias(v, k=8):
    return np.ascontiguousarray(np.asarray(v, np.float32).reshape(k, 128).T)


def _bf(a):
    return np.ascontiguousarray(np.asarray(a)).astype(ml_dtypes.bfloat16)


def _make_mask(j):
    q0s = (0, 1536) if j == 0 else (512, 1024)
    m = np.zeros((2, 8, 128, 512), np.float32)
    for sl in range(2):
        q0 = q0s[sl]
        for ki in range(8):
            kt = ki if sl == 0 else 8 + ki
            k0 = kt * 128
            i = np.arange(128)[:, None]
            jq = np.arange(512)[None, :]
            m[sl, ki] = ((q0 + jq) >= (k0 + i)).astype(np.float32)
    return m.astype(ml_dtypes.bfloat16)


def _make_wsel():
    w = np.zeros((4, 256), np.float32)
    for qb in range(2):
        for r in range(128):
            w[qb * 2 + (r // 64), qb * 128 + r] = 1.0
    return w.astype(ml_dtypes.bfloat16)


def kernel(**inputs):
    x = np.asarray(inputs["x"], np.float32)
    enc = np.asarray(inputs["encoder_output"], np.float32)
    shared = {}
    for name in ("wq1", "wk1", "wv1", "wo1", "wq2", "wk2", "wv2", "wo2",
                 "wf1", "wf2"):
        shared[name] = _bf(inputs[name])
    for src, dst in (("bq1", "cq1"), ("bk1", "ck1"), ("bo1", "co1"),
                     ("bq2", "cq2"), ("bk2", "ck2"), ("bo2", "co2"),
                     ("g1", "g1"), ("be1", "be1"), ("g2", "g2"), ("be2", "be2"),
                     ("g3", "g3"), ("be3", "be3")):
        shared[dst] = _colbias(inputs[src], 8)
    shared["cf1"] = _colbias(inputs["bf1"], 32)
    shared["cf2"] = _colbias(inputs["bf2"], 8)
    shared["rv1"] = _bf(np.asarray(inputs["bv1"]).reshape(1, D))
    shared["rv2"] = _bf(np.asarray(inputs["bv2"]).reshape(1, D))
    shared["rq1"] = _bf(np.asarray(inputs["bq1"]).reshape(1, D))
    shared["rk1"] = _bf(np.asarray(inputs["bk1"]).reshape(1, D))
    shared["rq2"] = _bf(np.asarray(inputs["bq2"]).reshape(1, D))
    shared["rk2"] = _bf(np.asarray(inputs["bk2"]).reshape(1, D))
    shared["wsel"] = _make_wsel()
    masks = {0: _make_mask(0), 1: _make_mask(1)}

    in_maps = []
    col_list = []
    for c in range(NCORES):
        b, j = c // 2, c % 2
        q0a, q0b = (0, 1536) if j == 0 else (512, 1024)
        cols = np.r_[q0a:q0a + 512, q0b:q0b + 512]
        col_list.append((b, cols))
        xTb = np.ascontiguousarray(x[b].T)
        encTb = np.ascontiguousarray(enc[b].T)
        m = dict(shared)
        m["xT"] = _bf(xTb)
        m["xTq"] = _bf(xTb[:, cols])
        m["encT"] = _bf(encTb)
        m["encTq"] = _bf(encTb[:, cols])
        m["bigmask"] = masks[j]
        in_maps.append(m)

    global _LAST_IN_MAPS
    _LAST_IN_MAPS = in_maps
    nc = _get_nc()
    res = run_bass_kernel_spmd(nc, in_maps, core_ids=list(range(NCORES)))
    out = np.empty((B, S, D), np.float32)
    for c in range(NCORES):
        b, cols = col_list[c]
        out[b, cols, :] = res.results[c]["outT"].T
    return out
